# revision 69
# baseline (speedup 1.0000x reference)
"""Trainium2 Bass kernel for nn_DINA_25503515804209 (sparse_attention).

Math (per batch b, head h):
  M = concat(x1, pad(x2)) in R^{2048 x 64}
  K = (1/8) * M U_h M^T          (2048 x 2048)
  rows_i = max(0, max_{p in allowed(i)} K[i,p])
  cols_p = max(0, max_{i in allowed(p)} K[i,p])
    (leading 848x848 block masked; the reference's mask fill value
     min(relu(K_head0)) is 0 for any real input since relu >= 0 and some
     entry is always <= 0 -- the max(0, .) floor implements it exactly)
  alpha = rows + cols; w1 = softmax(alpha[:1200]); w2 = softmax(alpha[1200:])
  r1 = w1 @ M[:1200]; r2 = w2 @ M[1200:]

Sharding: data-parallel over batch B=8 across the 8 NeuronCores.

Drain design (v3): three-engine split.  ACT copies each PSUM strip to
fp16 SBUF; the DVE computes per-strip row maxes with the 2-port TT-max
custom op over the copies; the column-max surface is built two ways in
parallel -- full strips t8..15 are reduced across partitions by the
(otherwise idle) Pool/GPSIMD engine with C-axis tensor_reduce into
partition-0 slots (gathered by one 8-descriptor DMA and finished with
tiny PE transposes), while strips t0..7 fold into an fp16 accumulator
via DVE tensor_max (finalized with PE transposes + one reduce).  The
boundary strip t6 uses a Relu(x + per-partition -inf bias) ACT copy to
mask its forbidden block, so no masked-reduce custom op is needed.
"""

import json

import numpy as np

B, L1, D1, L2, D2, H, C = 8, 1200, 64, 848, 48, 2, 64
Q = L1 + L2            # 2048
NT = Q // 128          # 16 row tiles
MASKED = L2            # leading 848x848 block is masked

_CACHE = {}


# --------------------------------------------------------------------------
# BIR post-processing: this walrus build encodes at most one semaphore wait
# per instruction; Tile emits multi-wait sync_infos.  Hoist excess waits
# into preceding same-engine EventSemaphore instructions (what wait_ge
# emits) -- engine sequencers execute in order, so semantics are identical.
# Also run codegen_inst_isa_subclasses, which populates .instr bytes for
# InstISA subclasses (custom DVE ops); raw Bass does not run that pass and
# walrus fails with "ISA wrong length" on empty instr arrays.
# --------------------------------------------------------------------------
def _split_waits_json(j):
    for fn in j.get("functions", []):
        for blk in fn.get("blocks", []):
            insts = blk.get("instructions")
            if not insts:
                continue
            out = []
            for ins in insts:
                si = ins.get("sync_info")
                waits = (si or {}).get("on_wait") or []
                if len(waits) > 1:
                    for k, wt in enumerate(waits[:-1]):
                        out.append(
                            {
                                "debug": ins.get("debug"),
                                "engine": ins["engine"],
                                "ins": [],
                                "name": f"{ins['name']}_hw{k}",
                                "opcode": "EventSemaphore",
                                "outs": [],
                                "sync_info": {"on_update": [], "on_wait": [wt]},
                            }
                        )
                    si["on_wait"] = waits[-1:]
                ups = (si or {}).get("on_update") or []
                if len(ups) > 1:
                    raise RuntimeError(
                        f"instruction {ins['name']} has {len(ups)} updates"
                    )
                out.append(ins)
            blk["instructions"] = out


def _patch_bass_json(nc):
    import concourse.mybir as mybir

    orig = nc.to_json_bytes
    done = []

    def to_json_bytes_patched():
        if not done:
            mybir.codegen_inst_isa_subclasses(nc)
            done.append(True)
        j = json.loads(orig())
        _split_waits_json(j)
        return json.dumps(j).encode()

    nc.to_json_bytes = to_json_bytes_patched
    return nc


def _ttmax_reduce_op():
    """Fused  out = max(in0, in1);  accum_out = rowmax(out)  custom DVE op.

    Consumes two fp16 streams per cycle (both DVE read ports), so one
    instruction replaces the whole pairwise row-max tree of a strip.
    Registered at runtime through dve_ops' documented extension point
    (the uop program ships in the per-NEFF DVE table)."""
    import numpy as np
    import concourse.dve_ops as dve_ops
    from concourse.dve_spec import Spec, Src0, Src1, maxx, lower
    from concourse.dve_table_gen import dve_ver_for
    from concourse.dve_uop import DveOpSpec

    NAME = "TT_MAX_ROWMAX_ANT"
    if NAME in dve_ops._SUB_OPCODE_FOR_NAME:
        return next(op for op in dve_ops.OPS if op.name == NAME)

    def _ref(in0, in1, c0, c1, c2):
        body = np.maximum(in0.astype(np.float32), in1.astype(np.float32))
        return body, body.reshape(body.shape[0], -1).max(axis=-1, keepdims=True)

    spec = Spec(body=maxx(Src0, Src1), accum=maxx, reference=_ref)
    row = dve_ops._CUSTOM_DVE_ROW_BASE + len(dve_ops.OPS)
    ver = dve_ver_for("TRN2")
    sha = DveOpSpec(
        name=NAME, opcode=row, uops=lower(spec, ver=ver), rd1_en=True
    ).sha(ver)
    op = dve_ops.DveOp(NAME, spec, subdim=False, uops_sha={ver: sha})
    dve_ops.OPS.append(op)
    dve_ops._SUB_OPCODE_FOR_NAME[NAME] = row
    dve_ops.CUSTOM_DVE_SPECS[NAME] = spec
    return op


def _build_nc():
    import concourse.bass as bass
    import concourse.mybir as mybir
    import concourse.tile as tile
    from concourse.dve_ops import TENSOR_MASK_REDUCE
    from concourse.masks import make_identity

    ttmax = _ttmax_reduce_op()

    f32 = mybir.dt.float32
    f32r = mybir.dt.float32r
    f16 = mybir.dt.float16
    AX = mybir.AxisListType
    ALU = mybir.AluOpType
    ACTF = mybir.ActivationFunctionType

    nc = bass.Bass(trn_type="TRN2")

    mt_d = nc.dram_tensor("mt_in", [C, Q], f32, kind="ExternalInput")
    m_d = nc.dram_tensor("m_in", [Q, C], f16, kind="ExternalInput")
    at_d = nc.dram_tensor("at_in", [C, 2, Q], f32, kind="ExternalInput")
    bv_d = nc.dram_tensor("bv_in", [128, 1], f32, kind="ExternalInput")
    bm_d = nc.dram_tensor("bmask_in", [128, 2], f32, kind="ExternalInput")
    out_d = nc.dram_tensor("out", [4, C], f32, kind="ExternalOutput")

    with tile.TileContext(nc) as tc:
        with (
            tc.tile_pool(name="sb", bufs=1) as sb,
            tc.tile_pool(name="escr", bufs=9) as escr,
        ):
            # ---- load inputs (f32r tiles loaded directly; PE rounds).
            # A^T = (M U_h)^T is precomputed on the host so the strip
            # matmuls start as soon as the first DMA chunks land.
            # Order matches T_ORDER: strip t0 (at chunk 0, mt 1..3) first.
            mtr = sb.tile([C, Q], f32r, tag="mtr")
            atr = sb.tile([C, 2, Q], f32r, tag="atr")
            # lhsT slices for the first strips (t0..t2, then t7) land first,
            # interleaved with the mt chunks they need; the at bulk follows
            nc.sync.dma_start(
                out=atr[:, :, 0:384], in_=at_d[:, :, 0:384].bitcast(f32r)
            )
            for j in (1, 2, 3):
                s = slice(512 * j, 512 * (j + 1))
                nc.sync.dma_start(out=mtr[:, s], in_=mt_d[:, s].bitcast(f32r))
            nc.sync.dma_start(
                out=atr[:, :, 896:1024], in_=at_d[:, :, 896:1024].bitcast(f32r)
            )
            nc.sync.dma_start(out=mtr[:, 0:512], in_=mt_d[:, 0:512].bitcast(f32r))
            for s in (slice(1024, 2048), slice(384, 896)):
                nc.sync.dma_start(out=atr[:, :, s], in_=at_d[:, :, s].bitcast(f32r))

            bv = sb.tile([128, 1], f32, tag="bv")
            nc.sync.dma_start(out=bv, in_=bv_d[:, :])

            ident16 = sb.tile([128, 128], f16, tag="ident16")
            make_identity(nc, ident16)
            ident32 = sb.tile([128, 128], f32, tag="ident32")
            make_identity(nc, ident32)

            rows0 = sb.tile([128, NT], f32, tag="rows0")
            rows1 = sb.tile([128, NT], f32, tag="rows1")
            cols0 = sb.tile([128, NT], f32, tag="cols0")
            cols1 = sb.tile([128, NT], f32, tag="cols1")
            colsT0 = sb.tile([128, NT], f32, tag="colsT0")
            colsT1 = sb.tile([128, NT], f32, tag="colsT1")
            colsT2_0 = sb.tile([128, NT], f32, tag="colsT2_0")
            colsT2_1 = sb.tile([128, NT], f32, tag="colsT2_1")
            r6b = sb.tile([128, 1], f32, tag="r6b")
            # accP: colmax accumulator for strips t0..7 (seeded by t7's copy)
            accP0 = sb.tile([128, Q], f16, tag="accP0")
            accP1 = sb.tile([128, Q], f16, tag="accP1")
            trA = sb.tile([128, Q // 2], f16, tag="trA")
            # Pool col-partials (strips t8..15) land on partition 0 (slot
            # t-8); one 8-descriptor DMA scatters them to T's partitions
            NPART = 8
            P0 = sb.tile([1, NPART, Q], f16, tag="P0")
            P1 = sb.tile([1, NPART, Q], f16, tag="P1")
            # T padded to 16 partitions for the xbar transpose (rows 8..15
            # are never read back: the stage-2 reduce slices slots 0:8)
            T0 = sb.tile([16, Q], f16, tag="T0")
            T1 = sb.tile([16, Q], f16, tag="T1")
            xA0 = sb.tile([128, NT, 128], f16, tag="xA0")
            xA1 = sb.tile([128, NT, 128], f16, tag="xA1")
            xT0 = sb.tile([128, NT, 16], f16, tag="xT0")
            xT1 = sb.tile([128, NT, 16], f16, tag="xT1")


            # softmax weights, interleaved for the single tail matmul group:
            # w4[:, t, 0:2] = seg1 (h0, h1), w4[:, t, 2:4] = seg2 (h0, h1)
            w4 = sb.tile([128, NT, 4], f16, tag="w4")
            nc.vector.memset(w4, 0.0)
            alpha_seg = sb.tile([128, 34], f32, tag="alpha_seg")
            s_pm = sb.tile([128, 4], f32, tag="s_pm")

            e2048 = sb.tile([128, 1], f32, tag="e2048")
            nc.vector.memset(e2048, float(Q))
            c7 = sb.tile([128, 1], f32, tag="c7")
            nc.vector.memset(c7, -7.0)

            # T rows 4:16 are read by the first partial xbar transpose
            # before being written; Pool initializes T while idle at start
            # (partition-offset memsets are rejected, so clear all rows)
            nc.gpsimd.memset(T0, 0.0)
            nc.gpsimd.memset(T1, 0.0)

            NRESTR = 6
            # Pool-fed strips alternate with accP-fed strips so the serial
            # Pool chain never bunches; t6 (boundary) right after the seed
            # so cols blocks 0:6 finalize mid-head; blocks 6:16 finalize
            # after the last restricted acc (t5).  t15 drains via DVE
            # maskreduce (ACT relief).
            T_ORDER = [0, 1, 2, 15, 7, 9, 6, 10, 3, 11, 12, 13, 14, 4, 8, 5]
            POOL_SLOT = {15: 0, 9: 1, 10: 2, 11: 3, 12: 4, 13: 5, 14: 6}
            with tc.tile_pool(name="psK", bufs=1, space="PSUM") as psK:
                def strips(h):
                    accP = accP0 if h == 0 else accP1
                    rows = rows0 if h == 0 else rows1
                    P = P0 if h == 0 else P1
                    cols = cols0 if h == 0 else cols1
                    colsT = colsT0 if h == 0 else colsT1
                    xA = xA0 if h == 0 else xA1
                    T = T0 if h == 0 else T1
                    xT = xT0 if h == 0 else xT1
                    pending = []   # accs deferred until accP is seeded (t7)
                    for ti, t in enumerate(T_ORDER):
                        isl = slice(128 * t, 128 * (t + 1))
                        lo = MASKED if t < NRESTR else 0
                        w = Q - lo
                        mmlo = 512 if t < NRESTR else 0
                        pkf = psK.tile([128, Q], f32, tag=f"pk{ti % 2}",
                                       name=f"pk_{h}_{t}")
                        pk = pkf[:, lo:Q]
                        for j in range(mmlo // 512, 4):
                            nc.tensor.matmul(
                                pkf[:, 512 * j : 512 * (j + 1)],
                                atr[:, h, isl],
                                mtr[:, 512 * j : 512 * (j + 1)],
                                start=True, stop=True,
                            )
                        if t == 15:
                            # DVE maskreduce drain (trivial full window):
                            # fp16 copy + rowmax in one op, freeing ACT;
                            # the copy feeds Pool's col-partial slot
                            e15 = escr.tile([128, Q], f16, tag="e",
                                            name=f"e15_{h}")
                            nc.vector._custom_dve(
                                TENSOR_MASK_REDUCE,
                                out=e15,
                                in0=pk[:, :],
                                in1=e2048,
                                s0=0.0,
                                s1=0.0,
                                imm2=1.0,
                                accum_out=rows[:, t : t + 1],
                            )
                            nc.gpsimd.tensor_reduce(
                                out=P[:, POOL_SLOT[t], :], in_=e15, axis=AX.C,
                                op=ALU.max,
                            )
                            continue
                        if t == NRESTR:
                            # boundary tile, two ACT copies: the forbidden
                            # [0:848] block is zeroed via Relu(x - inf_bias)
                            # (zeros are colmax-neutral; rows get the same
                            # floor from the final relu)
                            e6a = escr.tile([128, Q], f16, tag="e",
                                            name=f"e6a_{h}")[:, 0:MASKED]
                            nc.scalar.activation(
                                out=e6a, in_=pkf[:, 0:MASKED], func=ACTF.Relu,
                                bias=bv, scale=1.0,
                            )
                            e6b = escr.tile([128, Q], f16, tag="e",
                                            name=f"e6b_{h}")[:, 0 : Q - MASKED]
                            nc.scalar.copy(e6b, pkf[:, MASKED:Q])
                            nc.vector._custom_dve(
                                ttmax,
                                out=trA[:, 0:600],
                                in0=e6b[:, 0:600],
                                in1=e6b[:, 600:1200],
                                accum_out=rows[:, t : t + 1],
                            )
                            nc.vector._custom_dve(
                                ttmax,
                                out=trA[:, 0:424],
                                in0=e6a[:, 0:424],
                                in1=e6a[:, 424:848],
                                accum_out=r6b,
                            )
                            nc.vector.tensor_max(
                                rows[:, t : t + 1], rows[:, t : t + 1], r6b
                            )
                            nc.vector.tensor_max(
                                accP[:, 0:MASKED], accP[:, 0:MASKED], e6a
                            )
                            nc.vector.tensor_max(
                                accP[:, MASKED:Q], accP[:, MASKED:Q], e6b
                            )
                            continue
                        # ACT drains PSUM -> fp16; DVE row-maxes the copy
                        # via the 2-port TT-max tree
                        if t == 7:
                            e = accP[:, :]   # direct copy seeds accP
                        else:
                            e = escr.tile([128, Q], f16, tag="e",
                                          name=f"e_{t}_{h}")[:, 0:w]
                        nc.scalar.copy(e, pk[:, :])
                        nc.vector._custom_dve(
                            ttmax,
                            out=trA[:, 0 : w // 2],
                            in0=e[:, 0 : w // 2],
                            in1=e[:, w // 2 : w],
                            accum_out=rows[:, t : t + 1],
                        )
                        if t in POOL_SLOT:
                            # Pool col-partial: per-column max over the
                            # strip's 128 rows, into a partition-0 slot
                            k = POOL_SLOT[t]
                            nc.gpsimd.tensor_reduce(
                                out=P[:, k, :], in_=e, axis=AX.C, op=ALU.max
                            )
                            if k == 3:
                                # first four slots done: run the early half
                                # of the T-side finalize off the tail
                                nc.sync.dma_start(
                                    out=T[0:4, :], in_=P[0:1, 0:4, :]
                                )
                                nc.sync.dma_start_transpose(out=xT, in_=T)
                                nc.vector.tensor_reduce(
                                    out=colsT, in_=xT[:, :, 0:4],
                                    axis=AX.X, op=ALU.max,
                                )
                        elif t == 7:
                            for ep in pending:
                                nc.vector.tensor_max(
                                    accP[:, MASKED:Q], accP[:, MASKED:Q], ep,
                                )
                            pending = []
                        elif t in (0, 1, 2):
                            pending.append(e)   # accP not seeded yet
                        else:
                            nc.vector.tensor_max(
                                accP[:, lo:Q], accP[:, lo:Q], e
                            )
                            if t == 8:
                                # [0:768] sees only t7/e6a/t8: finalize
                                # cols blocks 0:6 now (t5 is restricted)
                                nc.sync.dma_start_transpose(
                                    out=xA[:, 0:6, :], in_=accP[:, 0:768]
                                )
                                nc.vector.tensor_reduce(
                                    out=cols[:, 0:6], in_=xA[:, 0:6, :],
                                    axis=AX.X, op=ALU.max,
                                )

                def finalize(h):
                    # Late half of the T-side finalize (slots 4:7), then the
                    # accP blocks 6:16 -- SP processes these in readiness
                    # order at the head's tail
                    colsT2 = colsT2_0 if h == 0 else colsT2_1
                    P = P0 if h == 0 else P1
                    T = T0 if h == 0 else T1
                    xT = xT0 if h == 0 else xT1
                    accP = accP0 if h == 0 else accP1
                    cols = cols0 if h == 0 else cols1
                    xA = xA0 if h == 0 else xA1
                    # partition-offset DMA writes are rejected; rewrite rows
                    # 0:7 (0:4 idempotent) after the early xbar consumed them
                    nc.sync.dma_start(out=T[0:7, :], in_=P[0:1, 0:7, :])
                    nc.sync.dma_start_transpose(out=xT, in_=T)
                    nc.vector.tensor_reduce(
                        out=colsT2, in_=xT[:, :, 4:7], axis=AX.X, op=ALU.max,
                    )
                    nc.sync.dma_start_transpose(
                        out=xA[:, 6:16, :], in_=accP[:, 768:Q]
                    )
                    nc.vector.tensor_reduce(
                        out=cols[:, 6:16], in_=xA[:, 6:16, :],
                        axis=AX.X, op=ALU.max,
                    )

                def head_relu(h):
                    cols = cols0 if h == 0 else cols1
                    colsT = colsT0 if h == 0 else colsT1
                    colsT2 = colsT2_0 if h == 0 else colsT2_1
                    rows = rows0 if h == 0 else rows1
                    # cols = relu(max(colsA, colsT, colsT2)), fused
                    nc.vector.scalar_tensor_tensor(
                        out=cols, in0=cols, scalar=0.0, in1=colsT,
                        op0=ALU.max, op1=ALU.max,
                    )
                    nc.vector.tensor_max(cols, cols, colsT2)
                    nc.vector.tensor_scalar_max(rows, rows, 0.0)

                def head_tail(h, bm):
                    # alpha (segment-aligned) and exp weights for head h;
                    # exp writes go straight into the interleaved w4 slots.
                    # Layout: [h0s1 0:10 | h1s1 10:20 | h0s2 20:27 | h1s2
                    # 27:34]; boundary row 1200 = tile 9 partition 48.
                    rows = rows0 if h == 0 else rows1
                    cols = cols0 if h == 0 else cols1
                    a1 = alpha_seg[:, 10 * h : 10 * h + 10]
                    a2 = alpha_seg[:, 20 + 7 * h : 27 + 7 * h]
                    nc.vector.tensor_add(a1, rows[:, 0:10], cols[:, 0:10])
                    nc.vector.tensor_add(a2, rows[:, 9:16], cols[:, 9:16])
                    # kill the out-of-segment halves of boundary tile 9 by
                    # adding -3e38 (host mask)
                    nc.vector.tensor_add(
                        a1[:, 9:10], a1[:, 9:10], bm[:, 0:1]
                    )
                    nc.vector.tensor_add(
                        a2[:, 0:1], a2[:, 0:1], bm[:, 1:2]
                    )
                    # alpha in [0, ~14): exp(alpha - 7) keeps the fp16
                    # weights in range; the constant shift cancels in the
                    # softmax normalization exactly
                    nc.scalar.activation(
                        out=w4[:, 0:10, h], in_=a1, func=ACTF.Exp,
                        bias=c7, scale=1.0, accum_out=s_pm[:, h : h + 1],
                    )
                    nc.scalar.activation(
                        out=w4[:, 9:16, 2 + h], in_=a2, func=ACTF.Exp,
                        bias=c7, scale=1.0, accum_out=s_pm[:, 2 + h : 3 + h],
                    )

                # late inputs (tail only), issued while strips run
                m_sb = sb.tile([128, NT, C], f16, tag="m_sb")
                bm = sb.tile([128, 2], f32, tag="bm")

                strips(0)
                finalize(0)
                nc.sync.dma_start(
                    out=m_sb,
                    in_=m_d[:, :].rearrange("(t p) c -> p t c", p=128),
                )
                nc.sync.dma_start(out=bm, in_=bm_d[:, :])
                strips(1)
                head_relu(0)
                head_tail(0, bm)
                finalize(1)
                head_relu(1)
                head_tail(1, bm)

            # ---- weighted-sum tail ----
            ssum = sb.tile([4, 1], f32, tag="ssum")
            srec = sb.tile([4, 1], f32, tag="srec")
            rt_sb = sb.tile([4, C], f32, tag="rt_sb")

            with tc.tile_pool(name="psF", bufs=1, space="PSUM") as psF:
                pm2 = psF.tile([128, 128], f32, tag="psmall", name="pm2")[0:4, :]
                nc.tensor.transpose(pm2[:, :], s_pm[:, :], ident32)
                nc.vector.tensor_reduce(out=ssum, in_=pm2[:, :], axis=AX.X, op=ALU.add)
                nc.vector.reciprocal(srec, ssum)

                # transposed accumulation: out[s, c] = sum_t,p w4[p,t,s] *
                # M[p,t,c] -- lands [4, C] directly, no copy/transpose after
                rp = psF.tile([4, C], f32, tag="rp")
                for t in range(NT):
                    nc.tensor.matmul(
                        rp[:, :], w4[:, t, :], m_sb[:, t, :],
                        start=(t == 0), stop=(t == NT - 1),
                    )
                nc.vector.tensor_scalar_mul(rt_sb, rp[:, :], srec)
                nc.sync.dma_start(out=out_d[:, :], in_=rt_sb)

    return nc


def _get_nc():
    if "nc" not in _CACHE:
        _CACHE["nc"] = _patch_bass_json(_build_nc())
    return _CACHE["nc"]


def _host_inputs(x1, x2, U):
    x1 = np.asarray(x1, dtype=np.float32)
    x2 = np.asarray(x2, dtype=np.float32)
    U = np.asarray(U, dtype=np.float32)
    us = (U * (C ** -0.5)).astype(np.float32)

    p = np.arange(128)
    # boundary tile 6 (rows 768:896): partitions p<80 are masked rows
    # (row < 848): -inf ACT-copy bias zeroes their [0:848] block via Relu
    bv = np.where(p < 80, -3.0e38, 0.0)[:, None].astype(np.float32)
    bm = np.zeros((128, 2), np.float32)
    bm[:, 0] = np.where(p >= L1 - 9 * 128, -3.0e38, 0.0)  # seg1 tile9: kill p>=48
    bm[:, 1] = np.where(p < L1 - 9 * 128, -3.0e38, 0.0)   # seg2 tile9: kill p<48

    in_maps = []
    for b in range(B):
        x2p = np.zeros((L2, C), np.float32)
        x2p[:, :D2] = x2[b]
        M = np.concatenate([x1[b], x2p], axis=0)  # [2048, 64]
        at = np.empty((C, 2, Q), np.float32)
        at[:, 0, :] = (M @ us[0]).T
        at[:, 1, :] = (M @ us[1]).T
        in_maps.append(
            {
                "mt_in": np.ascontiguousarray(M.T),
                "m_in": np.ascontiguousarray(M.astype(np.float16)),
                "at_in": at,
                "bv_in": bv,
                "bmask_in": bm,
            }
        )
    return in_maps


def run_cores(x1, x2, U, **kw):
    """Run on 8 cores; returns BassKernelResults."""
    from concourse.bass_utils import run_bass_kernel_spmd

    nc = _get_nc()
    in_maps = _host_inputs(x1, x2, U)
    return run_bass_kernel_spmd(nc, in_maps, core_ids=list(range(B)), **kw)


def kernel(x1, x2, U):
    res = run_cores(x1, x2, U)
    r1 = np.zeros((B, H, C), np.float32)
    r2 = np.zeros((B, H, C), np.float32)
    for b in range(B):
        o = res.results[b]["out"]
        r1[b] = o[0:2, :]
        r2[b] = o[2:4, :]
    return r1, r2


# revision 70
# speedup vs baseline: 1.0809x; 1.0809x over previous
"""Trainium2 Bass kernel for nn_DINA_25503515804209 (sparse_attention).

Math (per batch b, head h):
  M = concat(x1, pad(x2)) in R^{2048 x 64}
  K = (1/8) * M U_h M^T          (2048 x 2048)
  rows_i = max(0, max_{p in allowed(i)} K[i,p])
  cols_p = max(0, max_{i in allowed(p)} K[i,p])
    (leading 848x848 block masked; the reference's mask fill value
     min(relu(K_head0)) is 0 for any real input since relu >= 0 and some
     entry is always <= 0 -- the max(0, .) floor implements it exactly)
  alpha = rows + cols; w1 = softmax(alpha[:1200]); w2 = softmax(alpha[1200:])
  r1 = w1 @ M[:1200]; r2 = w2 @ M[1200:]

Sharding: data-parallel over batch B=8 across the 8 NeuronCores.

Drain design (v3): three-engine split.  ACT copies each PSUM strip to
fp16 SBUF; the DVE computes per-strip row maxes with the 2-port TT-max
custom op over the copies; the column-max surface is built two ways in
parallel -- full strips t8..15 are reduced across partitions by the
(otherwise idle) Pool/GPSIMD engine with C-axis tensor_reduce into
partition-0 slots (gathered by one 8-descriptor DMA and finished with
tiny PE transposes), while strips t0..7 fold into an fp16 accumulator
via DVE tensor_max (finalized with PE transposes + one reduce).  The
boundary strip t6 uses a Relu(x + per-partition -inf bias) ACT copy to
mask its forbidden block, so no masked-reduce custom op is needed.
"""

import json

import numpy as np

B, L1, D1, L2, D2, H, C = 8, 1200, 64, 848, 48, 2, 64
Q = L1 + L2            # 2048
NT = Q // 128          # 16 row tiles
MASKED = L2            # leading 848x848 block is masked

_CACHE = {}


# --------------------------------------------------------------------------
# BIR post-processing: this walrus build encodes at most one semaphore wait
# per instruction; Tile emits multi-wait sync_infos.  Hoist excess waits
# into preceding same-engine EventSemaphore instructions (what wait_ge
# emits) -- engine sequencers execute in order, so semantics are identical.
# Also run codegen_inst_isa_subclasses, which populates .instr bytes for
# InstISA subclasses (custom DVE ops); raw Bass does not run that pass and
# walrus fails with "ISA wrong length" on empty instr arrays.
# --------------------------------------------------------------------------
def _split_waits_json(j):
    for fn in j.get("functions", []):
        for blk in fn.get("blocks", []):
            insts = blk.get("instructions")
            if not insts:
                continue
            out = []
            for ins in insts:
                si = ins.get("sync_info")
                waits = (si or {}).get("on_wait") or []
                if len(waits) > 1:
                    for k, wt in enumerate(waits[:-1]):
                        out.append(
                            {
                                "debug": ins.get("debug"),
                                "engine": ins["engine"],
                                "ins": [],
                                "name": f"{ins['name']}_hw{k}",
                                "opcode": "EventSemaphore",
                                "outs": [],
                                "sync_info": {"on_update": [], "on_wait": [wt]},
                            }
                        )
                    si["on_wait"] = waits[-1:]
                ups = (si or {}).get("on_update") or []
                if len(ups) > 1:
                    raise RuntimeError(
                        f"instruction {ins['name']} has {len(ups)} updates"
                    )
                out.append(ins)
            blk["instructions"] = out


def _patch_bass_json(nc):
    import concourse.mybir as mybir

    orig = nc.to_json_bytes
    done = []

    def to_json_bytes_patched():
        if not done:
            mybir.codegen_inst_isa_subclasses(nc)
            done.append(True)
        j = json.loads(orig())
        _split_waits_json(j)
        return json.dumps(j).encode()

    nc.to_json_bytes = to_json_bytes_patched
    return nc


def _ttmax_reduce_op():
    """Fused  out = max(in0, in1);  accum_out = rowmax(out)  custom DVE op.

    Consumes two fp16 streams per cycle (both DVE read ports), so one
    instruction replaces the whole pairwise row-max tree of a strip.
    Registered at runtime through dve_ops' documented extension point
    (the uop program ships in the per-NEFF DVE table)."""
    import numpy as np
    import concourse.dve_ops as dve_ops
    from concourse.dve_spec import Spec, Src0, Src1, maxx, lower
    from concourse.dve_table_gen import dve_ver_for
    from concourse.dve_uop import DveOpSpec

    NAME = "TT_MAX_ROWMAX_ANT"
    if NAME in dve_ops._SUB_OPCODE_FOR_NAME:
        return next(op for op in dve_ops.OPS if op.name == NAME)

    def _ref(in0, in1, c0, c1, c2):
        body = np.maximum(in0.astype(np.float32), in1.astype(np.float32))
        return body, body.reshape(body.shape[0], -1).max(axis=-1, keepdims=True)

    spec = Spec(body=maxx(Src0, Src1), accum=maxx, reference=_ref)
    row = dve_ops._CUSTOM_DVE_ROW_BASE + len(dve_ops.OPS)
    ver = dve_ver_for("TRN2")
    sha = DveOpSpec(
        name=NAME, opcode=row, uops=lower(spec, ver=ver), rd1_en=True
    ).sha(ver)
    op = dve_ops.DveOp(NAME, spec, subdim=False, uops_sha={ver: sha})
    dve_ops.OPS.append(op)
    dve_ops._SUB_OPCODE_FOR_NAME[NAME] = row
    dve_ops.CUSTOM_DVE_SPECS[NAME] = spec
    return op


def _build_nc():
    import concourse.bass as bass
    import concourse.mybir as mybir
    import concourse.tile as tile
    from concourse.dve_ops import TENSOR_MASK_REDUCE
    from concourse.masks import make_identity

    ttmax = _ttmax_reduce_op()

    f32 = mybir.dt.float32
    f32r = mybir.dt.float32r
    f16 = mybir.dt.float16
    AX = mybir.AxisListType
    ALU = mybir.AluOpType
    ACTF = mybir.ActivationFunctionType

    nc = bass.Bass(trn_type="TRN2")

    mt_d = nc.dram_tensor("mt_in", [C, Q], f32, kind="ExternalInput")
    m_d = nc.dram_tensor("m_in", [Q, C], f16, kind="ExternalInput")
    at_d = nc.dram_tensor("at_in", [C, 2, Q], f32, kind="ExternalInput")
    bv_d = nc.dram_tensor("bv_in", [128, 1], f32, kind="ExternalInput")
    bm_d = nc.dram_tensor("bmask_in", [128, 2], f32, kind="ExternalInput")
    out_d = nc.dram_tensor("out", [4, C], f32, kind="ExternalOutput")

    with tile.TileContext(nc) as tc:
        with (
            tc.tile_pool(name="sb", bufs=1) as sb,
            tc.tile_pool(name="escr", bufs=9) as escr,
        ):
            # ---- load inputs (f32r tiles loaded directly; PE rounds).
            # A^T = (M U_h)^T is precomputed on the host so the strip
            # matmuls start as soon as the first DMA chunks land.
            # Order matches T_ORDER: strip t0 (at chunk 0, mt 1..3) first.
            mtr = sb.tile([C, Q], f32r, tag="mtr")
            atr = sb.tile([C, 2, Q], f32r, tag="atr")
            # lhsT slices for the first strips (t0..t2, then t7) land first,
            # interleaved with the mt chunks they need; the at bulk follows
            nc.sync.dma_start(
                out=atr[:, :, 0:384], in_=at_d[:, :, 0:384].bitcast(f32r)
            )
            for j in (1, 2, 3):
                s = slice(512 * j, 512 * (j + 1))
                nc.sync.dma_start(out=mtr[:, s], in_=mt_d[:, s].bitcast(f32r))
            nc.sync.dma_start(
                out=atr[:, :, 896:1024], in_=at_d[:, :, 896:1024].bitcast(f32r)
            )
            nc.sync.dma_start(out=mtr[:, 0:512], in_=mt_d[:, 0:512].bitcast(f32r))
            for s in (slice(1024, 2048), slice(384, 896)):
                nc.sync.dma_start(out=atr[:, :, s], in_=at_d[:, :, s].bitcast(f32r))

            bv = sb.tile([128, 1], f32, tag="bv")
            nc.sync.dma_start(out=bv, in_=bv_d[:, :])

            ident16 = sb.tile([128, 128], f16, tag="ident16")
            make_identity(nc, ident16)
            ident32 = sb.tile([128, 128], f32, tag="ident32")
            make_identity(nc, ident32)

            rows0 = sb.tile([128, NT], f32, tag="rows0")
            rows1 = sb.tile([128, NT], f32, tag="rows1")
            cols0 = sb.tile([128, NT], f32, tag="cols0")
            cols1 = sb.tile([128, NT], f32, tag="cols1")
            colsT0 = sb.tile([128, NT], f32, tag="colsT0")
            colsT1 = sb.tile([128, NT], f32, tag="colsT1")
            colsT2_0 = sb.tile([128, NT], f32, tag="colsT2_0")
            colsT2_1 = sb.tile([128, NT], f32, tag="colsT2_1")
            r6b = sb.tile([128, 1], f32, tag="r6b")
            # accP: colmax accumulator for strips t0..7 (seeded by t7's copy)
            accP0 = sb.tile([128, Q], f16, tag="accP0")
            accP1 = sb.tile([128, Q], f16, tag="accP1")
            trA = sb.tile([128, Q // 2], f16, tag="trA")
            # Pool col-partials (strips t8..15) land on partition 0 (slot
            # t-8); one 8-descriptor DMA scatters them to T's partitions
            NPART = 8
            P0 = sb.tile([1, NPART, Q], f16, tag="P0")
            P1 = sb.tile([1, NPART, Q], f16, tag="P1")
            # T padded to 16 partitions for the xbar transpose (rows 8..15
            # are never read back: the stage-2 reduce slices slots 0:8)
            T0 = sb.tile([16, Q], f16, tag="T0")
            T1 = sb.tile([16, Q], f16, tag="T1")
            xA0 = sb.tile([128, NT, 128], f16, tag="xA0")
            xA1 = sb.tile([128, NT, 128], f16, tag="xA1")
            xT0 = sb.tile([128, NT, 16], f16, tag="xT0")
            xT1 = sb.tile([128, NT, 16], f16, tag="xT1")


            # softmax weights, interleaved for the single tail matmul group:
            # w4[:, t, 0:2] = seg1 (h0, h1), w4[:, t, 2:4] = seg2 (h0, h1)
            w4 = sb.tile([128, NT, 4], f16, tag="w4")
            nc.vector.memset(w4, 0.0)
            alpha_seg = sb.tile([128, 34], f32, tag="alpha_seg")
            s_pm = sb.tile([128, 4], f32, tag="s_pm")

            e2048 = sb.tile([128, 1], f32, tag="e2048")
            nc.vector.memset(e2048, float(Q))
            c7 = sb.tile([128, 1], f32, tag="c7")
            nc.vector.memset(c7, -7.0)

            # T rows 4:16 are read by the first partial xbar transpose
            # before being written; Pool initializes T while idle at start
            # (partition-offset memsets are rejected, so clear all rows)
            nc.gpsimd.memset(T0, 0.0)
            nc.gpsimd.memset(T1, 0.0)

            NRESTR = 6
            # Pool-fed strips alternate with accP-fed strips so the serial
            # Pool chain never bunches; t6 (boundary) right after the seed
            # so cols blocks 0:6 finalize mid-head; blocks 6:16 finalize
            # after the last restricted acc (t5).  t15 drains via DVE
            # maskreduce (ACT relief).
            T_ORDER = [0, 1, 2, 15, 7, 9, 6, 10, 3, 11, 12, 13, 4, 14, 8, 5]
            POOL_SLOT = {15: 0, 9: 1, 10: 2, 11: 3, 12: 4, 13: 5, 14: 6, 8: 7}
            with tc.tile_pool(name="psK", bufs=1, space="PSUM") as psK:
                def strips(h):
                    accP = accP0 if h == 0 else accP1
                    rows = rows0 if h == 0 else rows1
                    P = P0 if h == 0 else P1
                    cols = cols0 if h == 0 else cols1
                    colsT = colsT0 if h == 0 else colsT1
                    xA = xA0 if h == 0 else xA1
                    T = T0 if h == 0 else T1
                    xT = xT0 if h == 0 else xT1
                    pending = []   # accs deferred until accP is seeded (t7)
                    for ti, t in enumerate(T_ORDER):
                        isl = slice(128 * t, 128 * (t + 1))
                        lo = MASKED if t < NRESTR else 0
                        w = Q - lo
                        mmlo = 512 if t < NRESTR else 0
                        pkf = psK.tile([128, Q], f32, tag=f"pk{ti % 2}",
                                       name=f"pk_{h}_{t}")
                        pk = pkf[:, lo:Q]
                        for j in range(mmlo // 512, 4):
                            nc.tensor.matmul(
                                pkf[:, 512 * j : 512 * (j + 1)],
                                atr[:, h, isl],
                                mtr[:, 512 * j : 512 * (j + 1)],
                                start=True, stop=True,
                            )
                        if t == 15:
                            # DVE maskreduce drain (trivial full window):
                            # fp16 copy + rowmax in one op, freeing ACT;
                            # the copy feeds Pool's col-partial slot
                            e15 = escr.tile([128, Q], f16, tag="e",
                                            name=f"e15_{h}")
                            nc.vector._custom_dve(
                                TENSOR_MASK_REDUCE,
                                out=e15,
                                in0=pk[:, :],
                                in1=e2048,
                                s0=0.0,
                                s1=0.0,
                                imm2=1.0,
                                accum_out=rows[:, t : t + 1],
                            )
                            nc.gpsimd.tensor_reduce(
                                out=P[:, POOL_SLOT[t], :], in_=e15, axis=AX.C,
                                op=ALU.max,
                            )
                            continue
                        if t == NRESTR:
                            # boundary tile, two ACT copies: the forbidden
                            # [0:848] block is zeroed via Relu(x - inf_bias)
                            # (zeros are colmax-neutral; rows get the same
                            # floor from the final relu)
                            e6a = escr.tile([128, Q], f16, tag="e",
                                            name=f"e6a_{h}")[:, 0:MASKED]
                            nc.scalar.activation(
                                out=e6a, in_=pkf[:, 0:MASKED], func=ACTF.Relu,
                                bias=bv, scale=1.0,
                            )
                            e6b = escr.tile([128, Q], f16, tag="e",
                                            name=f"e6b_{h}")[:, 0 : Q - MASKED]
                            nc.scalar.copy(e6b, pkf[:, MASKED:Q])
                            nc.vector._custom_dve(
                                ttmax,
                                out=trA[:, 0:600],
                                in0=e6b[:, 0:600],
                                in1=e6b[:, 600:1200],
                                accum_out=rows[:, t : t + 1],
                            )
                            nc.vector._custom_dve(
                                ttmax,
                                out=trA[:, 0:424],
                                in0=e6a[:, 0:424],
                                in1=e6a[:, 424:848],
                                accum_out=r6b,
                            )
                            nc.vector.tensor_max(
                                rows[:, t : t + 1], rows[:, t : t + 1], r6b
                            )
                            nc.vector.tensor_max(
                                accP[:, 0:MASKED], accP[:, 0:MASKED], e6a
                            )
                            nc.vector.tensor_max(
                                accP[:, MASKED:Q], accP[:, MASKED:Q], e6b
                            )
                            # cols blocks 0:6 ([0:768]) only see t7's seed
                            # and the boundary pieces: finalize them now
                            nc.sync.dma_start_transpose(
                                out=xA[:, 0:6, :], in_=accP[:, 0:768]
                            )
                            nc.vector.tensor_reduce(
                                out=cols[:, 0:6], in_=xA[:, 0:6, :],
                                axis=AX.X, op=ALU.max,
                            )
                            continue
                        # ACT drains PSUM -> fp16; DVE row-maxes the copy
                        # via the 2-port TT-max tree
                        if t == 7:
                            e = accP[:, :]   # direct copy seeds accP
                        else:
                            e = escr.tile([128, Q], f16, tag="e",
                                          name=f"e_{t}_{h}")[:, 0:w]
                        nc.scalar.copy(e, pk[:, :])
                        nc.vector._custom_dve(
                            ttmax,
                            out=trA[:, 0 : w // 2],
                            in0=e[:, 0 : w // 2],
                            in1=e[:, w // 2 : w],
                            accum_out=rows[:, t : t + 1],
                        )
                        if t >= 8:
                            # Pool col-partial: per-column max over the
                            # strip's 128 rows, into a partition-0 slot
                            k = POOL_SLOT[t]
                            nc.gpsimd.tensor_reduce(
                                out=P[:, k, :], in_=e, axis=AX.C, op=ALU.max
                            )
                            if k == 3:
                                # first four slots done: run the early half
                                # of the T-side finalize off the tail
                                nc.sync.dma_start(
                                    out=T[0:4, :], in_=P[0:1, 0:4, :]
                                )
                                nc.sync.dma_start_transpose(out=xT, in_=T)
                                nc.vector.tensor_reduce(
                                    out=colsT, in_=xT[:, :, 0:4],
                                    axis=AX.X, op=ALU.max,
                                )
                        elif t == 7:
                            for ep in pending:
                                nc.vector.tensor_max(
                                    accP[:, MASKED:Q], accP[:, MASKED:Q], ep,
                                )
                            pending = []
                        elif t in (0, 1, 2):
                            pending.append(e)   # accP not seeded yet
                        else:
                            nc.vector.tensor_max(
                                accP[:, lo:Q], accP[:, lo:Q], e
                            )
                            if t == 5:
                                # last restricted acc: finalize cols 6:16
                                nc.sync.dma_start_transpose(
                                    out=xA[:, 6:16, :], in_=accP[:, 768:Q]
                                )
                                nc.vector.tensor_reduce(
                                    out=cols[:, 6:16], in_=xA[:, 6:16, :],
                                    axis=AX.X, op=ALU.max,
                                )

                def finalize(h):
                    # Late half of the T-side finalize (slots 4:7), then the
                    # accP blocks 6:16 -- SP processes these in readiness
                    # order at the head's tail
                    colsT2 = colsT2_0 if h == 0 else colsT2_1
                    P = P0 if h == 0 else P1
                    T = T0 if h == 0 else T1
                    xT = xT0 if h == 0 else xT1
                    accP = accP0 if h == 0 else accP1
                    cols = cols0 if h == 0 else cols1
                    xA = xA0 if h == 0 else xA1
                    # partition-offset DMA writes are rejected; rewrite rows
                    # 0:8 (0:4 idempotent) after the early xbar consumed them
                    nc.sync.dma_start(out=T[0:NPART, :], in_=P[0:1, :, :])
                    nc.sync.dma_start_transpose(out=xT, in_=T)
                    nc.vector.tensor_reduce(
                        out=colsT2, in_=xT[:, :, 4:NPART], axis=AX.X, op=ALU.max,
                    )

                def head_relu(h):
                    cols = cols0 if h == 0 else cols1
                    colsT = colsT0 if h == 0 else colsT1
                    colsT2 = colsT2_0 if h == 0 else colsT2_1
                    rows = rows0 if h == 0 else rows1
                    # cols = relu(max(colsA, colsT, colsT2)), fused
                    nc.vector.scalar_tensor_tensor(
                        out=cols, in0=cols, scalar=0.0, in1=colsT,
                        op0=ALU.max, op1=ALU.max,
                    )
                    nc.vector.tensor_max(cols, cols, colsT2)
                    nc.vector.tensor_scalar_max(rows, rows, 0.0)

                def head_tail(h, bm):
                    # alpha (segment-aligned) and exp weights for head h;
                    # exp writes go straight into the interleaved w4 slots.
                    # Layout: [h0s1 0:10 | h1s1 10:20 | h0s2 20:27 | h1s2
                    # 27:34]; boundary row 1200 = tile 9 partition 48.
                    rows = rows0 if h == 0 else rows1
                    cols = cols0 if h == 0 else cols1
                    a1 = alpha_seg[:, 10 * h : 10 * h + 10]
                    a2 = alpha_seg[:, 20 + 7 * h : 27 + 7 * h]
                    nc.vector.tensor_add(a1, rows[:, 0:10], cols[:, 0:10])
                    nc.vector.tensor_add(a2, rows[:, 9:16], cols[:, 9:16])
                    # kill the out-of-segment halves of boundary tile 9 by
                    # adding -3e38 (host mask)
                    nc.vector.tensor_add(
                        a1[:, 9:10], a1[:, 9:10], bm[:, 0:1]
                    )
                    nc.vector.tensor_add(
                        a2[:, 0:1], a2[:, 0:1], bm[:, 1:2]
                    )
                    # alpha in [0, ~14): exp(alpha - 7) keeps the fp16
                    # weights in range; the constant shift cancels in the
                    # softmax normalization exactly
                    nc.scalar.activation(
                        out=w4[:, 0:10, h], in_=a1, func=ACTF.Exp,
                        bias=c7, scale=1.0, accum_out=s_pm[:, h : h + 1],
                    )
                    nc.scalar.activation(
                        out=w4[:, 9:16, 2 + h], in_=a2, func=ACTF.Exp,
                        bias=c7, scale=1.0, accum_out=s_pm[:, 2 + h : 3 + h],
                    )

                # late inputs (tail only), issued while strips run
                m_sb = sb.tile([128, NT, C], f16, tag="m_sb")
                bm = sb.tile([128, 2], f32, tag="bm")

                strips(0)
                finalize(0)
                nc.sync.dma_start(
                    out=m_sb,
                    in_=m_d[:, :].rearrange("(t p) c -> p t c", p=128),
                )
                nc.sync.dma_start(out=bm, in_=bm_d[:, :])
                strips(1)
                head_relu(0)
                head_tail(0, bm)
                finalize(1)
                head_relu(1)
                head_tail(1, bm)

            # ---- weighted-sum tail ----
            ssum = sb.tile([4, 1], f32, tag="ssum")
            srec = sb.tile([4, 1], f32, tag="srec")
            rt_sb = sb.tile([4, C], f32, tag="rt_sb")

            with tc.tile_pool(name="psF", bufs=1, space="PSUM") as psF:
                pm2 = psF.tile([128, 128], f32, tag="psmall", name="pm2")[0:4, :]
                nc.tensor.transpose(pm2[:, :], s_pm[:, :], ident32)
                nc.vector.tensor_reduce(out=ssum, in_=pm2[:, :], axis=AX.X, op=ALU.add)
                nc.vector.reciprocal(srec, ssum)

                # transposed accumulation: out[s, c] = sum_t,p w4[p,t,s] *
                # M[p,t,c] -- lands [4, C] directly, no copy/transpose after
                rp = psF.tile([4, C], f32, tag="rp")
                for t in range(NT):
                    nc.tensor.matmul(
                        rp[:, :], w4[:, t, :], m_sb[:, t, :],
                        start=(t == 0), stop=(t == NT - 1),
                    )
                nc.vector.tensor_scalar_mul(rt_sb, rp[:, :], srec)
                nc.sync.dma_start(out=out_d[:, :], in_=rt_sb)

    return nc


def _get_nc():
    if "nc" not in _CACHE:
        _CACHE["nc"] = _patch_bass_json(_build_nc())
    return _CACHE["nc"]


def _host_inputs(x1, x2, U):
    x1 = np.asarray(x1, dtype=np.float32)
    x2 = np.asarray(x2, dtype=np.float32)
    U = np.asarray(U, dtype=np.float32)
    us = (U * (C ** -0.5)).astype(np.float32)

    p = np.arange(128)
    # boundary tile 6 (rows 768:896): partitions p<80 are masked rows
    # (row < 848): -inf ACT-copy bias zeroes their [0:848] block via Relu
    bv = np.where(p < 80, -3.0e38, 0.0)[:, None].astype(np.float32)
    bm = np.zeros((128, 2), np.float32)
    bm[:, 0] = np.where(p >= L1 - 9 * 128, -3.0e38, 0.0)  # seg1 tile9: kill p>=48
    bm[:, 1] = np.where(p < L1 - 9 * 128, -3.0e38, 0.0)   # seg2 tile9: kill p<48

    in_maps = []
    for b in range(B):
        x2p = np.zeros((L2, C), np.float32)
        x2p[:, :D2] = x2[b]
        M = np.concatenate([x1[b], x2p], axis=0)  # [2048, 64]
        at = np.empty((C, 2, Q), np.float32)
        at[:, 0, :] = (M @ us[0]).T
        at[:, 1, :] = (M @ us[1]).T
        in_maps.append(
            {
                "mt_in": np.ascontiguousarray(M.T),
                "m_in": np.ascontiguousarray(M.astype(np.float16)),
                "at_in": at,
                "bv_in": bv,
                "bmask_in": bm,
            }
        )
    return in_maps


def run_cores(x1, x2, U, **kw):
    """Run on 8 cores; returns BassKernelResults."""
    from concourse.bass_utils import run_bass_kernel_spmd

    nc = _get_nc()
    in_maps = _host_inputs(x1, x2, U)
    return run_bass_kernel_spmd(nc, in_maps, core_ids=list(range(B)), **kw)


def kernel(x1, x2, U):
    res = run_cores(x1, x2, U)
    r1 = np.zeros((B, H, C), np.float32)
    r2 = np.zeros((B, H, C), np.float32)
    for b in range(B):
        o = res.results[b]["out"]
        r1[b] = o[0:2, :]
        r2[b] = o[2:4, :]
    return r1, r2
